# revision 37
# baseline (speedup 1.0000x reference)
"""Trainium2 Bass kernel for the GRU-GCN cell (nn_GRUCell).

Sharding: 8 NeuronCores, node-parallel (128 nodes/core, all 32 batches).
v2 design:
  - LN via closed-form stats (no u materialization), emb gathered in fp8
    (softmax is diagonal-dominant: off-diag mass < 3e-6, fp8 logit noise
    only perturbs e^-13-scale weights).
  - One 1024-wide exp per batch from a 2-bank PSUM logits tile.
  - exp recomputed in the upd phase (saves the 16MB DRAM exp spill).
  - Both weight-gens + k0 out-mm matmuls overlap the two AllGathers.
  - Prepacked [b,m,q,c] input layout -> 2KB/partition DMA lines.
  - partition_broadcast for softmax-denominator replication (no PSUM).
"""

import os
import sys

sys.path.insert(0, "/opt/trn_rl_repo")
import numpy as np

B, N, D = 32, 1024, 64
DI = DO = 64
C = DI + DO  # 128
OG, OU = 2 * DO, DO  # 128, 64
NCORES = 8
NL = N // NCORES  # 128
NG = NL // 4  # 32 groups of 4 nodes
EPS = 1e-12
LAG = 8  # upd-phase exp lookahead depth

_CACHE = {}
LAST_RESULT = None


def _np_reference(x, state, node_emb, time_emb, gate_w, gate_b, gate_gamma,
                  gate_beta, upd_w, upd_b, upd_gamma, upd_beta):
    """Plain numpy fallback (general layernorm parameters)."""

    def _ln(v, g, b2):
        mu = v.mean(-1, keepdims=True)
        var = ((v - mu) ** 2).mean(-1, keepdims=True)
        return (v - mu) / np.sqrt(var + EPS) * g + b2

    def _gcn(xg, w_pool, b_pool, g, b2):
        emb = _ln(node_emb[None] + time_emb[:, None], g, b2)
        logits = np.einsum("bnd,bmd->bnm", emb, emb, optimize=True)
        a = np.exp(logits - logits.max(-1, keepdims=True))
        a /= a.sum(-1, keepdims=True)
        xg2 = np.einsum("bnm,bmc->bnc", a, xg, optimize=True)
        w = np.einsum("nd,dkio->nkio", node_emb, w_pool, optimize=True)
        bias = time_emb @ b_pool
        return (np.einsum("bni,nio->bno", xg, w[:, 0], optimize=True)
                + np.einsum("bni,nio->bno", xg2, w[:, 1], optimize=True)
                + bias[:, None, :])

    inp = np.concatenate([x, state], -1)
    zr = 1.0 / (1.0 + np.exp(-_gcn(inp, gate_w, gate_b, gate_gamma, gate_beta)))
    z, r = zr[..., :DO], zr[..., DO:]
    cand = np.concatenate([x, z * state], -1)
    hc = np.tanh(_gcn(cand, upd_w, upd_b, upd_gamma, upd_beta))
    return (r * state + (1.0 - r) * hc).astype(np.float32)


def _install_prof_shim():
    import types

    if "antenv.axon_hooks" in sys.modules:
        return
    try:
        from trn_agent_boot.trn_boot import _ntff_profile_via_ctypes

        hook = _ntff_profile_via_ctypes("/opt/axon/libaxon_pjrt.so")
    except Exception:
        hook = None
    mod = types.ModuleType("antenv.axon_hooks")
    mod.get_axon_ntff_profile_hook = lambda: hook

    def _set(h):
        mod.get_axon_ntff_profile_hook = lambda: h

    mod.set_axon_ntff_profile_hook = _set
    sys.modules["antenv.axon_hooks"] = mod
    try:
        import antenv

        antenv.axon_hooks = mod
    except Exception:
        pass


def _build():
    import concourse.bacc as bacc
    import concourse.mybir as mybir
    from concourse.tile import TileContext
    from concourse.masks import make_identity

    F16 = mybir.dt.float16
    F32 = mybir.dt.float32
    F8 = mybir.dt.float8e4
    AF = mybir.ActivationFunctionType
    ALU = mybir.AluOpType

    nc = bacc.Bacc()

    def pin(name, shape, dt=F16):
        return nc.declare_dram_parameter(name, shape, dt, isOutput=False)

    ne_f32 = pin("ne_f32", [NL, D], F32)
    neT16 = pin("neT16", [D, NL])
    te_f32 = pin("te_f32", [B, D], F32)
    te16r = pin("te16r", [1, B * D])
    teT16 = pin("teT16", [D, B])
    teT4_16 = pin("teT4_16", [D, 4 * B])
    xq16 = pin("xq16", [B, 128, 8, C])       # prepacked (b, m, q, c)
    xTn16 = pin("xTn16", [DI, NL, B])        # c-major local x (n-major)
    stTn16 = pin("stTn16", [DO, NL, B])      # c-major local state (n-major)
    st_loc = pin("st_loc", [B, NL, DO], F32)
    pg16 = pin("pg16", [2, D, OG, C])        # (k, d, o, i)
    pu16 = pin("pu16", [2, D, OU, C])
    gb16 = pin("gb16", [D, OG])
    ub16 = pin("ub16", [D, OU])
    h_out = nc.declare_dram_parameter("h_out", [128, NG * DO], F32, isOutput=True)
    dbg = os.environ.get("KDBG")
    if dbg:
        zr_dbg = nc.declare_dram_parameter("zr_dbg", [128, NG * OG], F16, isOutput=True)
        hc_dbg = nc.declare_dram_parameter("hc_dbg", [128, NG * DO], F16, isOutput=True)
        xg2_dbg = nc.declare_dram_parameter("xg2_dbg", [C, B * NL], F16, isOutput=True)
        xg2u_dbg = nc.declare_dram_parameter("xg2u_dbg", [C, B * NL], F16, isOutput=True)
        cand_dbg = nc.declare_dram_parameter("cand_dbg", [C, B * NL], F16, isOutput=True)
        sinv_dbg = nc.declare_dram_parameter("sinv_dbg", [1, B * NL], F16, isOutput=True)

    with TileContext(nc) as tc:
        with (
            tc.tile_pool(name="const", bufs=1) as cpool,
            tc.tile_pool(name="big", bufs=1) as big,
            tc.tile_pool(name="stage", bufs=2) as stg,
            tc.tile_pool(name="dram", bufs=1, space="DRAM") as dram,
        ):
            # ---------- constants / persistent tiles (LN-critical first) ----
            ne_sb = cpool.tile([NL, D], F32, tag="ne_sb")
            nc.gpsimd.dma_start(ne_sb[:], ne_f32[:])
            neT_sb = cpool.tile([D, NL], F16, tag="neT_sb")
            nc.scalar.dma_start(neT_sb[:], neT16[:])
            teT_sb = cpool.tile([D, B], F16, tag="teT_sb")
            nc.sync.dma_start(teT_sb[:], teT16[:])
            te16_sb = cpool.tile([1, B * D], F16, tag="te16_sb")
            nc.gpsimd.dma_start(te16_sb[:], te16r[:])
            ones_col16 = cpool.tile([128, 1], F16, tag="ones_col16")
            nc.gpsimd.memset(ones_col16[:], 1.0)
            ones_row16 = cpool.tile([1, 128], F16, tag="ones_row16")
            nc.gpsimd.memset(ones_row16[:], 1.0)
            ident16 = cpool.tile([128, 128], F16, tag="ident16")
            make_identity(nc, ident16[:])
            neg64_col = cpool.tile([128, 1], F32, tag="neg64_col")
            nc.gpsimd.memset(neg64_col[:], -64.0)
            teT4_sb = cpool.tile([D, 4 * B], F16, tag="teT4_sb")
            nc.gpsimd.dma_start(teT4_sb[:], teT4_16[:])
            gb_sb = cpool.tile([D, OG], F16, tag="gb_sb")
            nc.gpsimd.dma_start(gb_sb[:], gb16[:])
            ub_sb = cpool.tile([D, OU], F16, tag="ub_sb")
            nc.gpsimd.dma_start(ub_sb[:], ub16[:])

            embT_loc8 = big.tile([D, B * NL], F8, tag="embT_loc8")
            embT_all8 = big.tile([D, NCORES, B, NL], F8, tag="embT_all8")
            xg2T = big.tile([C, B * NL], F16, tag="xg2T")
            xg2uT = big.tile([C, B * NL], F16, tag="xg2uT")
            inpT_cm = big.tile([C, B * NL], F16, tag="inpT_cm")
            candT = big.tile([C, B * NL], F16, tag="candT")
            zr_sb = big.tile([128, NG * OG], F16, tag="zr_sb")
            state_grp = big.tile([128, NG * DO], F32, tag="state_grp")
            zs_grp = big.tile([128, NG * DO], F16, tag="zs_grp")
            hc_sb = big.tile([128, NG * DO], F16, tag="hc_sb")
            sinv_sb = big.tile([1, B * NL], F16, tag="sinv_sb")
            biasg_rep = big.tile([128, OG], F32, tag="biasg_rep")
            biasu_rep = big.tile([128, OU], F32, tag="biasu_rep")

            # c-major inputs (SP engine: keep gpsimd free for LN broadcasts)
            nc.sync.dma_start(inpT_cm[0:DI, :], xTn16[:].rearrange("d n b -> d (n b)"))
            nc.sync.dma_start(inpT_cm[DI:C, :], stTn16[:].rearrange("d n b -> d (n b)"))
            nc.sync.dma_start(candT[0:DI, :], xTn16[:].rearrange("d n b -> d (n b)"))
            for jj in range(4):
                nc.sync.dma_start(
                    state_grp[32 * jj : 32 * jj + 32, :]
                    .rearrange("b (g o) -> b g o", o=DO),
                    st_loc[:].rearrange("b (g jj) o -> b g jj o", jj=4)[:, :, jj, :],
                )

            # DRAM scratch for collectives
            d_embT_in = dram.tile([D, B * NL], F8, tag="d_embT_in")
            d_embT_out = dram.tile([NCORES, D, B * NL], F8, tag="d_embT_out",
                                   addr_space="Shared")
            d_zs_in = dram.tile([NL, B * DO], F16, tag="d_zs_in")
            d_zs_out = dram.tile([NCORES, NL, B * DO], F16, tag="d_zs_out",
                                 addr_space="Shared")

            # ---------- bias ----------
            with nc.named_scope("bias"):
                with tc.tile_pool(name="psb", bufs=1, space="PSUM") as psb:
                    ps_bg = psb.tile([128, OG], F32, tag="ps_bg")
                    nc.tensor.matmul(ps_bg[:], teT4_sb[:], gb_sb[:], start=True, stop=True)
                    nc.vector.tensor_copy(biasg_rep[:], ps_bg[:])
                    ps_bu = psb.tile([128, OU], F32, tag="ps_bu")
                    nc.tensor.matmul(ps_bu[:], teT4_sb[:], ub_sb[:], start=True, stop=True)
                    nc.vector.tensor_copy(biasu_rep[:], ps_bu[:])

            # ---------- LN via stats ----------
            with nc.named_scope("LN"):
                with (
                    tc.tile_pool(name="ln", bufs=1) as ln,
                    tc.tile_pool(name="psl", bufs=1, space="PSUM") as psl,
                    tc.tile_pool(name="pse", bufs=2, space="PSUM") as pse,
                ):
                    # time-emb stats via matmuls (ones^T @ teT, ones^T @ teT^2)
                    teT2 = ln.tile([D, B], F16, tag="teT2")
                    nc.vector.tensor_mul(teT2[:], teT_sb[:], teT_sb[:])
                    ps_te = psl.tile([1, 2 * B], F32, tag="ps_te")
                    nc.tensor.matmul(ps_te[:, 0:B], ones_col16[0:D, :], teT_sb[:],
                                     start=True, stop=True)
                    nc.tensor.matmul(ps_te[:, B : 2 * B], ones_col16[0:D, :], teT2[:],
                                     start=True, stop=True)
                    te_st = ln.tile([1, 2 * B], F16, tag="te_st")
                    with nc.allow_low_precision("ln stats bcast in f16"):
                        nc.vector.tensor_scalar_mul(te_st[:], ps_te[:], 1.0 / D)
                    te_st_rep = psl.tile([128, 2 * B], F32, tag="te_st_rep")
                    nc.tensor.matmul(te_st_rep[:], ones_row16[:], te_st[:],
                                     start=True, stop=True)

                    # node-emb stats
                    ne2 = ln.tile([NL, D], F32, tag="ne2")
                    nc.vector.tensor_mul(ne2[:], ne_sb[:], ne_sb[:])
                    ne_mu = ln.tile([NL, 1], F32, tag="ne_mu")
                    nc.vector.reduce_sum(ne_mu[:], ne_sb[:], axis=mybir.AxisListType.X)
                    nc.vector.tensor_scalar_mul(ne_mu[:], ne_mu[:], 1.0 / D)
                    ne_sq = ln.tile([NL, 1], F32, tag="ne_sq")
                    nc.vector.reduce_sum(ne_sq[:], ne2[:], axis=mybir.AxisListType.X)
                    nc.vector.tensor_scalar_mul(ne_sq[:], ne_sq[:], 1.0 / D)

                    # cross term: ne @ te^T
                    ps_cross = psl.tile([NL, B], F32, tag="ps_cross")
                    nc.tensor.matmul(ps_cross[:], neT_sb[:], teT_sb[:],
                                     start=True, stop=True)

                    mu_all = ln.tile([NL, B], F32, tag="mu_all")
                    nc.vector.tensor_scalar(
                        out=mu_all[:], in0=te_st_rep[:, 0:B],
                        scalar1=ne_mu[:], scalar2=None,
                        op0=ALU.add,
                    )
                    ex2 = ln.tile([NL, B], F32, tag="ex2")
                    nc.vector.tensor_scalar(
                        out=ex2[:], in0=te_st_rep[:, B : 2 * B],
                        scalar1=ne_sq[:], scalar2=None,
                        op0=ALU.add,
                    )
                    nc.vector.scalar_tensor_tensor(
                        out=ex2[:], in0=ps_cross[:], scalar=2.0 / D, in1=ex2[:],
                        op0=ALU.mult, op1=ALU.add,
                    )
                    musq = ln.tile([NL, B], F32, tag="musq")
                    nc.vector.tensor_mul(musq[:], mu_all[:], mu_all[:])
                    var_all = ln.tile([NL, B], F32, tag="var_all")
                    nc.vector.tensor_sub(var_all[:], ex2[:], musq[:])
                    nc.vector.tensor_scalar_add(var_all[:], var_all[:], EPS)
                    sd_all = ln.tile([NL, B], F32, tag="sd_all")
                    nc.scalar.sqrt(sd_all[:], var_all[:])
                    rstd = ln.tile([NL, B], F32, tag="rstd")
                    nc.vector.reciprocal(rstd[:], sd_all[:])

                    for h in range(2):
                        te_rep = pse.tile([128, B * D // 2], F32, tag="te_rep",
                                          bufs=1)
                        for ch in range(2):
                            cs = slice(ch * 512, (ch + 1) * 512)
                            gs = slice(h * 1024 + ch * 512,
                                       h * 1024 + (ch + 1) * 512)
                            nc.tensor.matmul(te_rep[:, cs], ones_row16[:],
                                             te16_sb[:, gs], start=True, stop=True)
                        for b in range(h * B // 2, (h + 1) * B // 2):
                            bo = b - h * B // 2
                            eng = nc.vector
                            ub_t = stg.tile([NL, D], F32, tag="ub_t", bufs=3)
                            eng.scalar_tensor_tensor(
                                out=ub_t[:], in0=te_rep[:, bo * D : (bo + 1) * D],
                                scalar=mu_all[:, b : b + 1], in1=ne_sb[:],
                                op0=ALU.subtract, op1=ALU.add,
                            )
                            embn = stg.tile([NL, D], F16, tag="embn", bufs=3)
                            eng.tensor_scalar(
                                out=embn[:], in0=ub_t[:],
                                scalar1=rstd[:, b : b + 1], scalar2=None,
                                op0=ALU.mult,
                            )
                            ps_tr = pse.tile([D, NL], F16, tag="ps_tr")
                            nc.tensor.transpose(ps_tr[:], embn[:], ident16[:])
                            nc.vector.tensor_copy(
                                embT_loc8[:, b * NL : (b + 1) * NL], ps_tr[:]
                            )

            # ---------- AllGather emb (fp8) ----------
            nc.gpsimd.dma_start(d_embT_in[:], embT_loc8[:])
            nc.gpsimd.collective_compute(
                "AllGather",
                ALU.bypass,
                replica_groups=[list(range(NCORES))],
                ins=[d_embT_in.opt()],
                outs=[d_embT_out.opt()],
            )
            # gathered emb -> SBUF (q-major), split across two DMA queues
            for q in range(NCORES):
                eng = nc.sync if q % 2 == 0 else nc.gpsimd
                eng.dma_start(
                    embT_all8[:, q],
                    d_embT_out[q].rearrange("d (b n) -> d b n", n=NL),
                )

            # ---------- w-gen (overlaps AllGather) ----------
            def wgen(pool_dram, n_o, wslab, scope, dma_eng=None):
                dma_eng = dma_eng or nc.scalar
                OSZ = 16  # o's per pw load
                OB = 4    # o's per psum bank / copy
                engs = [nc.vector, nc.scalar]
                with nc.named_scope(scope):
                    with (
                        tc.tile_pool(name="psw", bufs=2, space="PSUM") as psw,
                        tc.tile_pool(name="pwstg", bufs=2) as pwstg,
                    ):
                        ei = 0
                        for k in range(2):
                            for oh in range(n_o // OSZ):
                                pw = pwstg.tile([D, OSZ * C], F16, tag="pw")
                                dma_eng.dma_start(
                                    pw[:],
                                    pool_dram[k, :, oh * OSZ : (oh + 1) * OSZ, :]
                                    .rearrange("d o i -> d (o i)"),
                                )
                                for ob in range(OSZ // OB):
                                    ps_w = psw.tile([C, OB * NL], F32, tag="ps_w")
                                    for oo in range(OB):
                                        o_in = ob * OB + oo
                                        nc.tensor.matmul(
                                            ps_w[:, oo * NL : (oo + 1) * NL],
                                            pw[:, o_in * C : (o_in + 1) * C],
                                            neT_sb[:], start=True, stop=True,
                                        )
                                    o0 = oh * OSZ + ob * OB
                                    col = (k * n_o + o0) * NL
                                    eng = engs[ei % 2]
                                    ei += 1
                                    if eng is nc.scalar:
                                        nc.scalar.activation(
                                            wslab[:, col : col + OB * NL],
                                            ps_w[:], AF.Copy,
                                        )
                                    else:
                                        eng.tensor_copy(
                                            wslab[:, col : col + OB * NL], ps_w[:]
                                        )

            with tc.tile_pool(name="wg", bufs=1) as wgp:
                wslab_g = wgp.tile([C, 2 * OG * NL], F16, tag="wslab_g")
                wgen(pg16, OG, wslab_g, "WGEN_G")

                # ---------- gate attention ----------
                with nc.named_scope("GATT"):
                    with (
                        tc.tile_pool(name="psg", bufs=1, space="PSUM") as psg,
                        tc.tile_pool(name="gat", bufs=1) as gat,
                    ):
                        _gat_ring = {}
                        for step in range(B + 2):
                            if step < B:
                                b = step
                                it_b = gat.tile([128, 8, C], F16, tag="it_b", bufs=4)
                                nc.sync.dma_start(it_b[:], xq16[b])
                                ps_log = psg.tile([128, 8 * NL], F32, tag="ps_log",
                                                  bufs=2)
                                for q in range(8):
                                    nc.tensor.matmul(
                                        ps_log[:, q * NL : (q + 1) * NL],
                                        embT_all8[:, q, b, :],
                                        embT_loc8[:, b * NL : (b + 1) * NL],
                                        start=True, stop=True,
                                    )
                                exp_sb = gat.tile([128, 8 * NL], F16, tag="exp_sb",
                                                  bufs=3)
                                nc.scalar.activation(exp_sb[:], ps_log[:], AF.Exp,
                                                     bias=neg64_col[:])
                                _gat_ring[b] = (it_b, exp_sb)
                            if step >= 2:
                                b = step - 2
                                it_p, exp_p = _gat_ring.pop(b)
                                ps_s = psg.tile([1, NL], F32, tag="ps_s", bufs=2)
                                ps_xg2 = psg.tile([C, NL], F32, tag="ps_xg2", bufs=2)
                                for q in range(8):
                                    nc.tensor.matmul(
                                        ps_s[:], ones_col16[:],
                                        exp_p[:, q * NL : (q + 1) * NL],
                                        start=(q == 0), stop=(q == 7),
                                    )
                                for q in range(8):
                                    nc.tensor.matmul(
                                        ps_xg2[:], it_p[:, q, :],
                                        exp_p[:, q * NL : (q + 1) * NL],
                                        start=(q == 0), stop=(q == 7),
                                    )
                                with nc.allow_low_precision("softmax scale in fp16"):
                                    nc.vector.reciprocal(
                                        sinv_sb[:, b * NL : (b + 1) * NL], ps_s[:]
                                    )
                                rep_sb = gat.tile([128, NL], F16, tag="rep_sb",
                                                  bufs=2)
                                nc.gpsimd.partition_broadcast(
                                    rep_sb[:], sinv_sb[:, b * NL : (b + 1) * NL]
                                )
                                nc.vector.tensor_mul(
                                    xg2T[:].rearrange("c (n b) -> c n b", b=B)[:, :, b],
                                    ps_xg2[:], rep_sb[:],
                                )

                # ---------- gate out-mm (+ sigmoid) ----------
                with nc.named_scope("GOUT"):
                    wview = wslab_g[:].rearrange("c (k o n) -> c k o n", k=2, o=OG)
                    with tc.tile_pool(name="pso", bufs=4, space="PSUM") as pso:
                        for g in range(NG):
                            ps_og = pso.tile([128, OG], F32, tag="og")
                            for jj in range(4):
                                n_ = g * 4 + jj
                                for ki, src in ((0, inpT_cm), (1, xg2T)):
                                    lhs = src[:, n_ * B : (n_ + 1) * B]
                                    rhs = wview[:, ki, :, n_]
                                    nc.tensor.matmul(
                                        ps_og[32 * jj : 32 * jj + 32, :],
                                        lhs, rhs,
                                        start=(ki == 0), stop=(ki == 1),
                                        tile_position=(0, 32 * jj),
                                    )
                            zt = stg.tile([128, OG], F32, tag="zt")
                            nc.vector.tensor_add(zt[:], ps_og[:], biasg_rep[:])
                            nc.scalar.activation(
                                zr_sb[:, g * OG : (g + 1) * OG], zt[:], AF.Sigmoid
                            )
                    nc.vector.tensor_mul(
                        zs_grp[:].rearrange("p (g o) -> p g o", o=DO),
                        zr_sb[:].rearrange("p (g o) -> p g o", o=OG)[:, :, 0:DO],
                        state_grp[:].rearrange("p (g o) -> p g o", o=DO),
                    )

            # zs -> dram rows [node | (b,o)] and AllGather
            for jj in range(4):
                nc.gpsimd.dma_start(
                    d_zs_in[:]
                    .rearrange("(g jj) (b o) -> jj b g o", jj=4, o=DO)[jj],
                    zs_grp[32 * jj : 32 * jj + 32, :]
                    .rearrange("b (g o) -> b g o", o=DO),
                )
            nc.gpsimd.collective_compute(
                "AllGather",
                ALU.bypass,
                replica_groups=[list(range(NCORES))],
                ins=[d_zs_in.opt()],
                outs=[d_zs_out.opt()],
            )

            # ---------- overlap zs AllGather ----------
            with tc.tile_pool(name="wu", bufs=1) as wup:
                wslab_u = wup.tile([C, 2 * OU * NL], F16, tag="wslab_u")

                with nc.named_scope("ZPREP"):
                    # candT rows 64:128 = (z*state)^T local
                    with tc.tile_pool(name="psz", bufs=2, space="PSUM") as psz:
                        for g in range(NG):
                            ps_zt = psz.tile([DO, 128], F16, tag="ps_zt")
                            nc.tensor.transpose(
                                ps_zt[:], zs_grp[:, g * DO : (g + 1) * DO], ident16[:]
                            )
                            nc.vector.tensor_copy(
                                candT[DI:C, (g * 4) * B : (g * 4 + 4) * B],
                                ps_zt[:],
                            )
                    nc.vector.tensor_copy(xg2uT[0:DI, :], xg2T[0:DI, :])

                wgen(pu16, OU, wslab_u, "WGEN_U")
                wuview = wslab_u[:].rearrange("c (k o n) -> c k o n", k=2, o=OU)

                with tc.tile_pool(name="upd", bufs=1) as updp:
                    # ---------- upd attention (recompute exp) + PV ----------
                    with nc.named_scope("UPV"):
                        with tc.tile_pool(name="psu", bufs=1, space="PSUM") as psu:
                            exp_ring = [None] * B
                            for i in range(B + LAG):
                                # PV(i-LAG) first: its exp_ring slot is about
                                # to be recycled by exp_u(i) below (bufs=LAG).
                                if i >= LAG:
                                    b = i - LAG
                                    if b % 4 == 0:
                                        zs_b4 = updp.tile([128, 8, 4 * DO], F16,
                                                          tag="zs_b4", bufs=2)
                                        nc.sync.dma_start(
                                            zs_b4[:],
                                            d_zs_out[:, :, b * DO : (b + 4) * DO]
                                            .rearrange("q m o -> m q o"),
                                        )
                                        _zs_cur = zs_b4
                                    ps_xu = psu.tile([DO, NL], F32, tag="ps_xu",
                                                     bufs=2)
                                    exp_b = exp_ring[b]
                                    bi = b % 4
                                    for q in range(8):
                                        nc.tensor.matmul(
                                            ps_xu[:],
                                            _zs_cur[:, q, bi * DO : (bi + 1) * DO],
                                            exp_b[:, q * NL : (q + 1) * NL],
                                            start=(q == 0), stop=(q == 7),
                                        )
                                    rep_u = updp.tile([128, NL], F16, tag="rep_u",
                                                      bufs=2)
                                    nc.gpsimd.partition_broadcast(
                                        rep_u[:], sinv_sb[:, b * NL : (b + 1) * NL]
                                    )
                                    nc.vector.tensor_mul(
                                        xg2uT[DI:C, :]
                                        .rearrange("c (n b) -> c n b", b=B)[:, :, b],
                                        ps_xu[:], rep_u[0:DO, :],
                                    )
                                if i < B:
                                    b = i
                                    ps_logu = psu.tile([128, 8 * NL], F32,
                                                       tag="ps_logu", bufs=2)
                                    for q in range(8):
                                        nc.tensor.matmul(
                                            ps_logu[:, q * NL : (q + 1) * NL],
                                            embT_all8[:, q, b, :],
                                            embT_loc8[:, b * NL : (b + 1) * NL],
                                            start=True, stop=True,
                                        )
                                    exp_u = updp.tile([128, 8 * NL], F16,
                                                      tag="exp_u", bufs=LAG)
                                    nc.scalar.activation(exp_u[:], ps_logu[:],
                                                         AF.Exp, bias=neg64_col[:])
                                    exp_ring[b] = exp_u

                    # ---------- upd out-mm + tanh ----------
                    with nc.named_scope("UOUT"), \
                            tc.tile_pool(name="psa", bufs=1, space="PSUM") as psa:
                        ps_all = psa.tile([128, NG * OU], F32, tag="ps_all")
                        t1_sb = updp.tile([128, NG * DO], F32, tag="t1_sb")
                        for g in range(NG):
                            for jj in range(4):
                                n_ = g * 4 + jj
                                for ki, src in ((0, candT), (1, xg2uT)):
                                    lhs = src[:, n_ * B : (n_ + 1) * B]
                                    nc.tensor.matmul(
                                        ps_all[32 * jj : 32 * jj + 32,
                                               g * OU : (g + 1) * OU],
                                        lhs, wuview[:, ki, :, n_],
                                        start=(ki == 0), stop=(ki == 1),
                                        tile_position=(0, 32 * jj),
                                    )
                            tt = stg.tile([128, OU], F32, tag="tt")
                            nc.vector.tensor_add(
                                tt[:], ps_all[:, g * OU : (g + 1) * OU],
                                biasu_rep[:],
                            )
                            nc.scalar.activation(
                                hc_sb[:, g * OU : (g + 1) * OU], tt[:], AF.Tanh
                            )
                            if g % 8 == 7:
                                g0 = g - 7
                                cs = slice(g0 * DO, (g + 1) * DO)
                                eng = nc.gpsimd
                                eng.tensor_sub(t1_sb[:, cs], state_grp[:, cs],
                                               hc_sb[:, cs])
                                eng.tensor_mul(
                                    t1_sb[:, cs].rearrange(
                                        "p (g o) -> p g o", o=DO),
                                    t1_sb[:, cs].rearrange(
                                        "p (g o) -> p g o", o=DO),
                                    zr_sb[:, g0 * OG : (g + 1) * OG]
                                    .rearrange("p (g o) -> p g o", o=OG)
                                    [:, :, DO:OG],
                                )
                                eng.tensor_add(t1_sb[:, cs], t1_sb[:, cs],
                                               hc_sb[:, cs])
                                nc.gpsimd.dma_start(h_out[:, cs], t1_sb[:, cs])
                if dbg:
                    nc.gpsimd.dma_start(zr_dbg[:], zr_sb[:])
                    nc.gpsimd.dma_start(hc_dbg[:], hc_sb[:])
                    nc.gpsimd.dma_start(xg2_dbg[:], xg2T[:])
                    nc.gpsimd.dma_start(xg2u_dbg[:], xg2uT[:])
                    nc.gpsimd.dma_start(cand_dbg[:], candT[:])
                    nc.gpsimd.dma_start(sinv_dbg[:], sinv_sb[:])

    nc.finalize()
    return nc


def _get_nc():
    key = "nc" + os.environ.get("KDBG", "")
    if key not in _CACHE:
        _CACHE[key] = _build()
    return _CACHE[key]


def kernel(x, state, node_emb, time_emb, gate_w, gate_b, gate_gamma, gate_beta,
           upd_w, upd_b, upd_gamma, upd_beta):
    global LAST_RESULT
    x = np.asarray(x, np.float32)
    state = np.asarray(state, np.float32)
    node_emb = np.asarray(node_emb, np.float32)
    time_emb = np.asarray(time_emb, np.float32)
    gate_w = np.asarray(gate_w, np.float32)
    gate_b = np.asarray(gate_b, np.float32)
    upd_w = np.asarray(upd_w, np.float32)
    upd_b = np.asarray(upd_b, np.float32)

    shared = (
        np.array_equal(np.asarray(gate_gamma), np.ones(D, np.float32))
        and np.array_equal(np.asarray(upd_gamma), np.ones(D, np.float32))
        and np.array_equal(np.asarray(gate_beta), np.zeros(D, np.float32))
        and np.array_equal(np.asarray(upd_beta), np.zeros(D, np.float32))
    )
    if not shared:
        return _np_reference(x, state, node_emb, time_emb, gate_w, gate_b,
                             gate_gamma, gate_beta, upd_w, upd_b, upd_gamma,
                             upd_beta)

    if os.environ.get("BASS_TRACE"):
        _install_prof_shim()

    from concourse.bass_utils import run_bass_kernel_spmd

    nc = _get_nc()

    xs = np.concatenate([x, state], axis=-1)  # [B, N, C]
    xq16 = np.ascontiguousarray(
        xs.reshape(B, 8, 128, C).transpose(0, 2, 1, 3)
    ).astype(np.float16)
    xTn16 = np.ascontiguousarray(x.transpose(2, 1, 0)).astype(np.float16)
    stTn16 = np.ascontiguousarray(state.transpose(2, 1, 0)).astype(np.float16)
    neT16 = np.ascontiguousarray(node_emb.T).astype(np.float16)
    teT16 = np.ascontiguousarray(time_emb.T).astype(np.float16)
    pg16 = np.ascontiguousarray(gate_w.transpose(1, 0, 3, 2)).astype(np.float16)
    pu16 = np.ascontiguousarray(upd_w.transpose(1, 0, 3, 2)).astype(np.float16)

    in_maps = []
    for c in range(NCORES):
        nlo = c * NL
        in_maps.append({
            "ne_f32": np.ascontiguousarray(node_emb[nlo : nlo + NL]),
            "neT16": np.ascontiguousarray(neT16[:, nlo : nlo + NL]),
            "te_f32": time_emb,
            "te16r": time_emb.reshape(1, B * D).astype(np.float16),
            "teT16": teT16,
            "teT4_16": np.tile(teT16, (1, 4)),
            "xq16": xq16,
            "xTn16": np.ascontiguousarray(xTn16[:, nlo : nlo + NL, :]),
            "stTn16": np.ascontiguousarray(stTn16[:, nlo : nlo + NL, :]),
            "st_loc": np.ascontiguousarray(state[:, nlo : nlo + NL, :]),
            "pg16": pg16,
            "pu16": pu16,
            "gb16": gate_b.astype(np.float16),
            "ub16": upd_b.astype(np.float16),
        })

    res = run_bass_kernel_spmd(
        nc, in_maps, list(range(NCORES)),
        trace=bool(os.environ.get("BASS_TRACE")),
    )
    LAST_RESULT = res

    h = np.empty((B, N, DO), np.float32)
    for c in range(NCORES):
        ho = res.results[c]["h_out"].reshape(4, 32, NG, DO)  # [jj, b, g, o]
        h[:, c * NL : (c + 1) * NL, :] = (
            ho.transpose(1, 2, 0, 3).reshape(B, NL, DO)
        )
    return h


# revision 38
# speedup vs baseline: 1.1629x; 1.1629x over previous
"""Trainium2 Bass kernel for the GRU-GCN cell (nn_GRUCell).

Sharding: 8 NeuronCores, node-parallel (128 nodes/core, all 32 batches).
v2 design:
  - LN via closed-form stats (no u materialization), emb gathered in fp8
    (softmax is diagonal-dominant: off-diag mass < 3e-6, fp8 logit noise
    only perturbs e^-13-scale weights).
  - One 1024-wide exp per batch from a 2-bank PSUM logits tile.
  - exp recomputed in the upd phase (saves the 16MB DRAM exp spill).
  - Both weight-gens + k0 out-mm matmuls overlap the two AllGathers.
  - Prepacked [b,m,q,c] input layout -> 2KB/partition DMA lines.
  - partition_broadcast for softmax-denominator replication (no PSUM).
"""

import os
import sys

sys.path.insert(0, "/opt/trn_rl_repo")
import numpy as np

B, N, D = 32, 1024, 64
DI = DO = 64
C = DI + DO  # 128
OG, OU = 2 * DO, DO  # 128, 64
NCORES = 8
NL = N // NCORES  # 128
NG = NL // 4  # 32 groups of 4 nodes
EPS = 1e-12
LAG = 8  # upd-phase exp lookahead depth

_CACHE = {}
LAST_RESULT = None


def _np_reference(x, state, node_emb, time_emb, gate_w, gate_b, gate_gamma,
                  gate_beta, upd_w, upd_b, upd_gamma, upd_beta):
    """Plain numpy fallback (general layernorm parameters)."""

    def _ln(v, g, b2):
        mu = v.mean(-1, keepdims=True)
        var = ((v - mu) ** 2).mean(-1, keepdims=True)
        return (v - mu) / np.sqrt(var + EPS) * g + b2

    def _gcn(xg, w_pool, b_pool, g, b2):
        emb = _ln(node_emb[None] + time_emb[:, None], g, b2)
        logits = np.einsum("bnd,bmd->bnm", emb, emb, optimize=True)
        a = np.exp(logits - logits.max(-1, keepdims=True))
        a /= a.sum(-1, keepdims=True)
        xg2 = np.einsum("bnm,bmc->bnc", a, xg, optimize=True)
        w = np.einsum("nd,dkio->nkio", node_emb, w_pool, optimize=True)
        bias = time_emb @ b_pool
        return (np.einsum("bni,nio->bno", xg, w[:, 0], optimize=True)
                + np.einsum("bni,nio->bno", xg2, w[:, 1], optimize=True)
                + bias[:, None, :])

    inp = np.concatenate([x, state], -1)
    zr = 1.0 / (1.0 + np.exp(-_gcn(inp, gate_w, gate_b, gate_gamma, gate_beta)))
    z, r = zr[..., :DO], zr[..., DO:]
    cand = np.concatenate([x, z * state], -1)
    hc = np.tanh(_gcn(cand, upd_w, upd_b, upd_gamma, upd_beta))
    return (r * state + (1.0 - r) * hc).astype(np.float32)


def _install_prof_shim():
    import types

    if "antenv.axon_hooks" in sys.modules:
        return
    try:
        from trn_agent_boot.trn_boot import _ntff_profile_via_ctypes

        hook = _ntff_profile_via_ctypes("/opt/axon/libaxon_pjrt.so")
    except Exception:
        hook = None
    mod = types.ModuleType("antenv.axon_hooks")
    mod.get_axon_ntff_profile_hook = lambda: hook

    def _set(h):
        mod.get_axon_ntff_profile_hook = lambda: h

    mod.set_axon_ntff_profile_hook = _set
    sys.modules["antenv.axon_hooks"] = mod
    try:
        import antenv

        antenv.axon_hooks = mod
    except Exception:
        pass


def _build():
    import concourse.bacc as bacc
    import concourse.mybir as mybir
    from concourse.tile import TileContext
    from concourse.masks import make_identity

    F16 = mybir.dt.float16
    F32 = mybir.dt.float32
    F8 = mybir.dt.float8e4
    AF = mybir.ActivationFunctionType
    ALU = mybir.AluOpType

    nc = bacc.Bacc()

    def pin(name, shape, dt=F16):
        return nc.declare_dram_parameter(name, shape, dt, isOutput=False)

    ne_f32 = pin("ne_f32", [NL, D], F32)
    neT16 = pin("neT16", [D, NL])
    te_f32 = pin("te_f32", [B, D], F32)
    te16r = pin("te16r", [1, B * D])
    teT16 = pin("teT16", [D, B])
    teT4_16 = pin("teT4_16", [D, 4 * B])
    xq16 = pin("xq16", [B, 128, 8, C])       # prepacked (b, m, q, c)
    xTn16 = pin("xTn16", [DI, NL, B])        # c-major local x (n-major)
    stTn16 = pin("stTn16", [DO, NL, B])      # c-major local state (n-major)
    st_loc = pin("st_loc", [B, NL, DO], F32)
    pg16 = pin("pg16", [2, D, OG, C])        # (k, d, o, i)
    pu16 = pin("pu16", [2, D, OU, C])
    gb16 = pin("gb16", [D, OG])
    ub16 = pin("ub16", [D, OU])
    h_out = nc.declare_dram_parameter("h_out", [128, NG * DO], F32, isOutput=True)
    dbg = os.environ.get("KDBG")
    if dbg:
        zr_dbg = nc.declare_dram_parameter("zr_dbg", [128, NG * OG], F16, isOutput=True)
        hc_dbg = nc.declare_dram_parameter("hc_dbg", [128, NG * DO], F16, isOutput=True)
        xg2_dbg = nc.declare_dram_parameter("xg2_dbg", [C, B * NL], F16, isOutput=True)
        xg2u_dbg = nc.declare_dram_parameter("xg2u_dbg", [C, B * NL], F16, isOutput=True)
        cand_dbg = nc.declare_dram_parameter("cand_dbg", [C, B * NL], F16, isOutput=True)
        sinv_dbg = nc.declare_dram_parameter("sinv_dbg", [1, B * NL], F16, isOutput=True)

    with TileContext(nc) as tc:
        with (
            tc.tile_pool(name="const", bufs=1) as cpool,
            tc.tile_pool(name="big", bufs=1) as big,
            tc.tile_pool(name="stage", bufs=2) as stg,
            tc.tile_pool(name="dram", bufs=1, space="DRAM") as dram,
        ):
            # ---------- constants / persistent tiles (LN-critical first) ----
            ne_sb = cpool.tile([NL, D], F32, tag="ne_sb")
            nc.gpsimd.dma_start(ne_sb[:], ne_f32[:])
            neT_sb = cpool.tile([D, NL], F16, tag="neT_sb")
            nc.gpsimd.dma_start(neT_sb[:], neT16[:])
            teT_sb = cpool.tile([D, B], F16, tag="teT_sb")
            nc.gpsimd.dma_start(teT_sb[:], teT16[:])
            te16_sb = cpool.tile([1, B * D], F16, tag="te16_sb")
            nc.gpsimd.dma_start(te16_sb[:], te16r[:])
            ones_col16 = cpool.tile([128, 1], F16, tag="ones_col16")
            nc.gpsimd.memset(ones_col16[:], 1.0)
            ones_row16 = cpool.tile([1, 128], F16, tag="ones_row16")
            nc.gpsimd.memset(ones_row16[:], 1.0)
            ident16 = cpool.tile([128, 128], F16, tag="ident16")
            make_identity(nc, ident16[:])
            neg64_col = cpool.tile([128, 1], F32, tag="neg64_col")
            nc.gpsimd.memset(neg64_col[:], -64.0)
            teT4_sb = cpool.tile([D, 4 * B], F16, tag="teT4_sb")
            nc.gpsimd.dma_start(teT4_sb[:], teT4_16[:])
            gb_sb = cpool.tile([D, OG], F16, tag="gb_sb")
            nc.gpsimd.dma_start(gb_sb[:], gb16[:])
            ub_sb = cpool.tile([D, OU], F16, tag="ub_sb")
            nc.gpsimd.dma_start(ub_sb[:], ub16[:])

            embT_loc8 = big.tile([D, B * NL], F8, tag="embT_loc8")
            embT_all8 = big.tile([D, NCORES, B, NL], F8, tag="embT_all8")
            xg2T = big.tile([C, B * NL], F16, tag="xg2T")
            xg2uT = big.tile([C, B * NL], F16, tag="xg2uT")
            inpT_cm = big.tile([C, B * NL], F16, tag="inpT_cm")
            candT = big.tile([C, B * NL], F16, tag="candT")
            zr_sb = big.tile([128, NG * OG], F16, tag="zr_sb")
            state_grp = big.tile([128, NG * DO], F32, tag="state_grp")
            zs_grp = big.tile([128, NG * DO], F16, tag="zs_grp")
            hc_sb = big.tile([128, NG * DO], F16, tag="hc_sb")
            sinv_sb = big.tile([1, B * NL], F16, tag="sinv_sb")
            biasg_rep = big.tile([128, OG], F32, tag="biasg_rep")
            biasu_rep = big.tile([128, OU], F32, tag="biasu_rep")

            # c-major inputs (SP engine: keep gpsimd free for LN broadcasts)
            nc.sync.dma_start(inpT_cm[0:DI, :], xTn16[:].rearrange("d n b -> d (n b)"))
            nc.sync.dma_start(inpT_cm[DI:C, :], stTn16[:].rearrange("d n b -> d (n b)"))
            nc.sync.dma_start(candT[0:DI, :], xTn16[:].rearrange("d n b -> d (n b)"))
            for jj in range(4):
                nc.sync.dma_start(
                    state_grp[32 * jj : 32 * jj + 32, :]
                    .rearrange("b (g o) -> b g o", o=DO),
                    st_loc[:].rearrange("b (g jj) o -> b g jj o", jj=4)[:, :, jj, :],
                )

            # DRAM scratch for collectives
            d_embT_in = dram.tile([D, B * NL], F8, tag="d_embT_in")
            d_embT_out = dram.tile([NCORES, D, B * NL], F8, tag="d_embT_out",
                                   addr_space="Shared")
            d_zs_in = dram.tile([NL, B * DO], F16, tag="d_zs_in")
            d_zs_out = dram.tile([NCORES, NL, B * DO], F16, tag="d_zs_out",
                                 addr_space="Shared")

            # ---------- bias ----------
            with nc.named_scope("bias"):
                with tc.tile_pool(name="psb", bufs=1, space="PSUM") as psb:
                    ps_bg = psb.tile([128, OG], F32, tag="ps_bg")
                    nc.tensor.matmul(ps_bg[:], teT4_sb[:], gb_sb[:], start=True, stop=True)
                    nc.vector.tensor_copy(biasg_rep[:], ps_bg[:])
                    ps_bu = psb.tile([128, OU], F32, tag="ps_bu")
                    nc.tensor.matmul(ps_bu[:], teT4_sb[:], ub_sb[:], start=True, stop=True)
                    nc.vector.tensor_copy(biasu_rep[:], ps_bu[:])

            # ---------- LN via stats ----------
            with nc.named_scope("LN"):
                with (
                    tc.tile_pool(name="ln", bufs=1) as ln,
                    tc.tile_pool(name="psl", bufs=1, space="PSUM") as psl,
                    tc.tile_pool(name="pse", bufs=2, space="PSUM") as pse,
                ):
                    # time-emb stats via matmuls (ones^T @ teT, ones^T @ teT^2)
                    teT2 = ln.tile([D, B], F16, tag="teT2")
                    nc.vector.tensor_mul(teT2[:], teT_sb[:], teT_sb[:])
                    ps_te = psl.tile([1, 2 * B], F32, tag="ps_te")
                    nc.tensor.matmul(ps_te[:, 0:B], ones_col16[0:D, :], teT_sb[:],
                                     start=True, stop=True)
                    nc.tensor.matmul(ps_te[:, B : 2 * B], ones_col16[0:D, :], teT2[:],
                                     start=True, stop=True)
                    te_st = ln.tile([1, 2 * B], F16, tag="te_st")
                    with nc.allow_low_precision("ln stats bcast in f16"):
                        nc.vector.tensor_scalar_mul(te_st[:], ps_te[:], 1.0 / D)
                    te_st_rep = psl.tile([128, 2 * B], F32, tag="te_st_rep")
                    nc.tensor.matmul(te_st_rep[:], ones_row16[:], te_st[:],
                                     start=True, stop=True)

                    # node-emb stats
                    ne2 = ln.tile([NL, D], F32, tag="ne2")
                    nc.vector.tensor_mul(ne2[:], ne_sb[:], ne_sb[:])
                    ne_mu = ln.tile([NL, 1], F32, tag="ne_mu")
                    nc.vector.reduce_sum(ne_mu[:], ne_sb[:], axis=mybir.AxisListType.X)
                    nc.vector.tensor_scalar_mul(ne_mu[:], ne_mu[:], 1.0 / D)
                    ne_sq = ln.tile([NL, 1], F32, tag="ne_sq")
                    nc.vector.reduce_sum(ne_sq[:], ne2[:], axis=mybir.AxisListType.X)
                    nc.vector.tensor_scalar_mul(ne_sq[:], ne_sq[:], 1.0 / D)

                    # cross term: ne @ te^T
                    ps_cross = psl.tile([NL, B], F32, tag="ps_cross")
                    nc.tensor.matmul(ps_cross[:], neT_sb[:], teT_sb[:],
                                     start=True, stop=True)

                    mu_all = ln.tile([NL, B], F32, tag="mu_all")
                    nc.vector.tensor_scalar(
                        out=mu_all[:], in0=te_st_rep[:, 0:B],
                        scalar1=ne_mu[:], scalar2=None,
                        op0=ALU.add,
                    )
                    ex2 = ln.tile([NL, B], F32, tag="ex2")
                    nc.vector.tensor_scalar(
                        out=ex2[:], in0=te_st_rep[:, B : 2 * B],
                        scalar1=ne_sq[:], scalar2=None,
                        op0=ALU.add,
                    )
                    nc.vector.scalar_tensor_tensor(
                        out=ex2[:], in0=ps_cross[:], scalar=2.0 / D, in1=ex2[:],
                        op0=ALU.mult, op1=ALU.add,
                    )
                    musq = ln.tile([NL, B], F32, tag="musq")
                    nc.vector.tensor_mul(musq[:], mu_all[:], mu_all[:])
                    var_all = ln.tile([NL, B], F32, tag="var_all")
                    nc.vector.tensor_sub(var_all[:], ex2[:], musq[:])
                    nc.vector.tensor_scalar_add(var_all[:], var_all[:], EPS)
                    sd_all = ln.tile([NL, B], F32, tag="sd_all")
                    nc.scalar.sqrt(sd_all[:], var_all[:])
                    rstd = ln.tile([NL, B], F32, tag="rstd")
                    nc.vector.reciprocal(rstd[:], sd_all[:])

                    for h in range(2):
                        te_rep = pse.tile([128, B * D // 2], F32, tag="te_rep",
                                          bufs=1)
                        for ch in range(2):
                            cs = slice(ch * 512, (ch + 1) * 512)
                            gs = slice(h * 1024 + ch * 512,
                                       h * 1024 + (ch + 1) * 512)
                            nc.tensor.matmul(te_rep[:, cs], ones_row16[:],
                                             te16_sb[:, gs], start=True, stop=True)
                        for b in range(h * B // 2, (h + 1) * B // 2):
                            bo = b - h * B // 2
                            eng = nc.vector
                            ub_t = stg.tile([NL, D], F32, tag="ub_t", bufs=3)
                            eng.scalar_tensor_tensor(
                                out=ub_t[:], in0=te_rep[:, bo * D : (bo + 1) * D],
                                scalar=mu_all[:, b : b + 1], in1=ne_sb[:],
                                op0=ALU.subtract, op1=ALU.add,
                            )
                            embn = stg.tile([NL, D], F16, tag="embn", bufs=3)
                            eng.tensor_scalar(
                                out=embn[:], in0=ub_t[:],
                                scalar1=rstd[:, b : b + 1], scalar2=None,
                                op0=ALU.mult,
                            )
                            ps_tr = pse.tile([D, NL], F16, tag="ps_tr")
                            nc.tensor.transpose(ps_tr[:], embn[:], ident16[:])
                            nc.vector.tensor_copy(
                                embT_loc8[:, b * NL : (b + 1) * NL], ps_tr[:]
                            )

            # ---------- AllGather emb (fp8) ----------
            nc.gpsimd.dma_start(d_embT_in[:], embT_loc8[:])
            nc.gpsimd.collective_compute(
                "AllGather",
                ALU.bypass,
                replica_groups=[list(range(NCORES))],
                ins=[d_embT_in.opt()],
                outs=[d_embT_out.opt()],
            )
            # gathered emb -> SBUF (q-major), split across two DMA queues
            for q in range(NCORES):
                eng = nc.sync if q % 2 == 0 else nc.gpsimd
                eng.dma_start(
                    embT_all8[:, q],
                    d_embT_out[q].rearrange("d (b n) -> d b n", n=NL),
                )

            # ---------- w-gen (overlaps AllGather) ----------
            def wgen(pool_dram, n_o, wslab, scope, dma_eng=None):
                dma_eng = dma_eng or nc.scalar
                OSZ = 16  # o's per pw load
                OB = 4    # o's per psum bank / copy
                engs = [nc.vector, nc.scalar]
                with nc.named_scope(scope):
                    with (
                        tc.tile_pool(name="psw", bufs=2, space="PSUM") as psw,
                        tc.tile_pool(name="pwstg", bufs=2) as pwstg,
                    ):
                        ei = 0
                        for k in range(2):
                            for oh in range(n_o // OSZ):
                                pw = pwstg.tile([D, OSZ * C], F16, tag="pw")
                                dma_eng.dma_start(
                                    pw[:],
                                    pool_dram[k, :, oh * OSZ : (oh + 1) * OSZ, :]
                                    .rearrange("d o i -> d (o i)"),
                                )
                                for ob in range(OSZ // OB):
                                    ps_w = psw.tile([C, OB * NL], F32, tag="ps_w")
                                    for oo in range(OB):
                                        o_in = ob * OB + oo
                                        nc.tensor.matmul(
                                            ps_w[:, oo * NL : (oo + 1) * NL],
                                            pw[:, o_in * C : (o_in + 1) * C],
                                            neT_sb[:], start=True, stop=True,
                                        )
                                    o0 = oh * OSZ + ob * OB
                                    col = (k * n_o + o0) * NL
                                    eng = engs[ei % 2]
                                    ei += 1
                                    if eng is nc.scalar:
                                        nc.scalar.activation(
                                            wslab[:, col : col + OB * NL],
                                            ps_w[:], AF.Copy,
                                        )
                                    else:
                                        eng.tensor_copy(
                                            wslab[:, col : col + OB * NL], ps_w[:]
                                        )

            with tc.tile_pool(name="wg", bufs=1) as wgp:
                wslab_g = wgp.tile([C, 2 * OG * NL], F16, tag="wslab_g")
                wgen(pg16, OG, wslab_g, "WGEN_G")

                # ---------- gate attention ----------
                with nc.named_scope("GATT"):
                    with (
                        tc.tile_pool(name="psg", bufs=1, space="PSUM") as psg,
                        tc.tile_pool(name="gat", bufs=1) as gat,
                    ):
                        _gat_ring = {}
                        for step in range(B + 2):
                            if step < B:
                                b = step
                                it_b = gat.tile([128, 8, C], F16, tag="it_b", bufs=4)
                                nc.sync.dma_start(it_b[:], xq16[b])
                                ps_log = psg.tile([128, 8 * NL], F32, tag="ps_log",
                                                  bufs=2)
                                for q in range(8):
                                    nc.tensor.matmul(
                                        ps_log[:, q * NL : (q + 1) * NL],
                                        embT_all8[:, q, b, :],
                                        embT_loc8[:, b * NL : (b + 1) * NL],
                                        start=True, stop=True,
                                    )
                                exp_sb = gat.tile([128, 8 * NL], F16, tag="exp_sb",
                                                  bufs=3)
                                nc.scalar.activation(exp_sb[:], ps_log[:], AF.Exp,
                                                     bias=neg64_col[:])
                                _gat_ring[b] = (it_b, exp_sb)
                            if step >= 2:
                                b = step - 2
                                it_p, exp_p = _gat_ring.pop(b)
                                ps_s = psg.tile([1, NL], F32, tag="ps_s", bufs=2)
                                ps_xg2 = psg.tile([C, NL], F32, tag="ps_xg2", bufs=2)
                                for q in range(8):
                                    nc.tensor.matmul(
                                        ps_s[:], ones_col16[:],
                                        exp_p[:, q * NL : (q + 1) * NL],
                                        start=(q == 0), stop=(q == 7),
                                    )
                                for q in range(8):
                                    nc.tensor.matmul(
                                        ps_xg2[:], it_p[:, q, :],
                                        exp_p[:, q * NL : (q + 1) * NL],
                                        start=(q == 0), stop=(q == 7),
                                    )
                                with nc.allow_low_precision("softmax scale in fp16"):
                                    nc.vector.reciprocal(
                                        sinv_sb[:, b * NL : (b + 1) * NL], ps_s[:]
                                    )
                                rep_sb = gat.tile([128, NL], F16, tag="rep_sb",
                                                  bufs=2)
                                nc.gpsimd.partition_broadcast(
                                    rep_sb[:], sinv_sb[:, b * NL : (b + 1) * NL]
                                )
                                nc.vector.tensor_mul(
                                    xg2T[:].rearrange("c (n b) -> c n b", b=B)[:, :, b],
                                    ps_xg2[:], rep_sb[:],
                                )

                # ---------- gate out-mm (+ sigmoid) ----------
                with nc.named_scope("GOUT"):
                    wview = wslab_g[:].rearrange("c (k o n) -> c k o n", k=2, o=OG)
                    with tc.tile_pool(name="pso", bufs=4, space="PSUM") as pso:
                        for g in range(NG):
                            ps_og = pso.tile([128, OG], F32, tag="og")
                            for jj in range(4):
                                n_ = g * 4 + jj
                                for ki, src in ((0, inpT_cm), (1, xg2T)):
                                    lhs = src[:, n_ * B : (n_ + 1) * B]
                                    rhs = wview[:, ki, :, n_]
                                    nc.tensor.matmul(
                                        ps_og[32 * jj : 32 * jj + 32, :],
                                        lhs, rhs,
                                        start=(ki == 0), stop=(ki == 1),
                                        tile_position=(0, 32 * jj),
                                    )
                            zt = stg.tile([128, OG], F32, tag="zt")
                            nc.vector.tensor_add(zt[:], ps_og[:], biasg_rep[:])
                            nc.scalar.activation(
                                zr_sb[:, g * OG : (g + 1) * OG], zt[:], AF.Sigmoid
                            )
                    nc.vector.tensor_mul(
                        zs_grp[:].rearrange("p (g o) -> p g o", o=DO),
                        zr_sb[:].rearrange("p (g o) -> p g o", o=OG)[:, :, 0:DO],
                        state_grp[:].rearrange("p (g o) -> p g o", o=DO),
                    )

            # zs -> dram rows [node | (b,o)] and AllGather
            for jj in range(4):
                nc.gpsimd.dma_start(
                    d_zs_in[:]
                    .rearrange("(g jj) (b o) -> jj b g o", jj=4, o=DO)[jj],
                    zs_grp[32 * jj : 32 * jj + 32, :]
                    .rearrange("b (g o) -> b g o", o=DO),
                )
            nc.gpsimd.collective_compute(
                "AllGather",
                ALU.bypass,
                replica_groups=[list(range(NCORES))],
                ins=[d_zs_in.opt()],
                outs=[d_zs_out.opt()],
            )

            # ---------- overlap zs AllGather ----------
            with tc.tile_pool(name="wu", bufs=1) as wup:
                wslab_u = wup.tile([C, 2 * OU * NL], F16, tag="wslab_u")

                with nc.named_scope("ZPREP"):
                    # candT rows 64:128 = (z*state)^T local
                    with tc.tile_pool(name="psz", bufs=2, space="PSUM") as psz:
                        for g in range(NG):
                            ps_zt = psz.tile([DO, 128], F16, tag="ps_zt")
                            nc.tensor.transpose(
                                ps_zt[:], zs_grp[:, g * DO : (g + 1) * DO], ident16[:]
                            )
                            nc.vector.tensor_copy(
                                candT[DI:C, (g * 4) * B : (g * 4 + 4) * B],
                                ps_zt[:],
                            )
                    nc.vector.tensor_copy(xg2uT[0:DI, :], xg2T[0:DI, :])

                wgen(pu16, OU, wslab_u, "WGEN_U")
                wuview = wslab_u[:].rearrange("c (k o n) -> c k o n", k=2, o=OU)

                with tc.tile_pool(name="upd", bufs=1) as updp:
                    # ---------- upd attention (recompute exp) + PV ----------
                    with nc.named_scope("UPV"):
                        with tc.tile_pool(name="psu", bufs=1, space="PSUM") as psu:
                            exp_ring = [None] * B
                            for i in range(B + LAG):
                                # PV(i-LAG) first: its exp_ring slot is about
                                # to be recycled by exp_u(i) below (bufs=LAG).
                                if i >= LAG:
                                    b = i - LAG
                                    if b % 4 == 0:
                                        zs_b4 = updp.tile([128, 8, 4 * DO], F16,
                                                          tag="zs_b4", bufs=2)
                                        nc.sync.dma_start(
                                            zs_b4[:],
                                            d_zs_out[:, :, b * DO : (b + 4) * DO]
                                            .rearrange("q m o -> m q o"),
                                        )
                                        _zs_cur = zs_b4
                                    ps_xu = psu.tile([DO, NL], F32, tag="ps_xu",
                                                     bufs=2)
                                    exp_b = exp_ring[b]
                                    bi = b % 4
                                    for q in range(8):
                                        nc.tensor.matmul(
                                            ps_xu[:],
                                            _zs_cur[:, q, bi * DO : (bi + 1) * DO],
                                            exp_b[:, q * NL : (q + 1) * NL],
                                            start=(q == 0), stop=(q == 7),
                                        )
                                    rep_u = updp.tile([128, NL], F16, tag="rep_u",
                                                      bufs=2)
                                    nc.gpsimd.partition_broadcast(
                                        rep_u[:], sinv_sb[:, b * NL : (b + 1) * NL]
                                    )
                                    nc.vector.tensor_mul(
                                        xg2uT[DI:C, :]
                                        .rearrange("c (n b) -> c n b", b=B)[:, :, b],
                                        ps_xu[:], rep_u[0:DO, :],
                                    )
                                if i < B:
                                    b = i
                                    ps_logu = psu.tile([128, 8 * NL], F32,
                                                       tag="ps_logu", bufs=2)
                                    for q in range(8):
                                        nc.tensor.matmul(
                                            ps_logu[:, q * NL : (q + 1) * NL],
                                            embT_all8[:, q, b, :],
                                            embT_loc8[:, b * NL : (b + 1) * NL],
                                            start=True, stop=True,
                                        )
                                    exp_u = updp.tile([128, 8 * NL], F16,
                                                      tag="exp_u", bufs=LAG)
                                    nc.scalar.activation(exp_u[:], ps_logu[:],
                                                         AF.Exp, bias=neg64_col[:])
                                    exp_ring[b] = exp_u

                    # ---------- upd out-mm + tanh ----------
                    with nc.named_scope("UOUT"), \
                            tc.tile_pool(name="psa", bufs=1, space="PSUM") as psa:
                        ps_all = psa.tile([128, NG * OU], F32, tag="ps_all")
                        t1_sb = updp.tile([128, NG * DO], F32, tag="t1_sb")
                        for g in range(NG):
                            for jj in range(4):
                                n_ = g * 4 + jj
                                for ki, src in ((0, candT), (1, xg2uT)):
                                    lhs = src[:, n_ * B : (n_ + 1) * B]
                                    nc.tensor.matmul(
                                        ps_all[32 * jj : 32 * jj + 32,
                                               g * OU : (g + 1) * OU],
                                        lhs, wuview[:, ki, :, n_],
                                        start=(ki == 0), stop=(ki == 1),
                                        tile_position=(0, 32 * jj),
                                    )
                            tt = stg.tile([128, OU], F32, tag="tt")
                            nc.vector.tensor_add(
                                tt[:], ps_all[:, g * OU : (g + 1) * OU],
                                biasu_rep[:],
                            )
                            nc.scalar.activation(
                                hc_sb[:, g * OU : (g + 1) * OU], tt[:], AF.Tanh
                            )
                            if g % 8 == 7:
                                g0 = g - 7
                                cs = slice(g0 * DO, (g + 1) * DO)
                                eng = nc.gpsimd
                                eng.tensor_sub(t1_sb[:, cs], state_grp[:, cs],
                                               hc_sb[:, cs])
                                eng.tensor_mul(
                                    t1_sb[:, cs].rearrange(
                                        "p (g o) -> p g o", o=DO),
                                    t1_sb[:, cs].rearrange(
                                        "p (g o) -> p g o", o=DO),
                                    zr_sb[:, g0 * OG : (g + 1) * OG]
                                    .rearrange("p (g o) -> p g o", o=OG)
                                    [:, :, DO:OG],
                                )
                                eng.tensor_add(t1_sb[:, cs], t1_sb[:, cs],
                                               hc_sb[:, cs])
                                nc.gpsimd.dma_start(h_out[:, cs], t1_sb[:, cs])
                if dbg:
                    nc.gpsimd.dma_start(zr_dbg[:], zr_sb[:])
                    nc.gpsimd.dma_start(hc_dbg[:], hc_sb[:])
                    nc.gpsimd.dma_start(xg2_dbg[:], xg2T[:])
                    nc.gpsimd.dma_start(xg2u_dbg[:], xg2uT[:])
                    nc.gpsimd.dma_start(cand_dbg[:], candT[:])
                    nc.gpsimd.dma_start(sinv_dbg[:], sinv_sb[:])

    nc.finalize()
    return nc


def _get_nc():
    key = "nc" + os.environ.get("KDBG", "")
    if key not in _CACHE:
        _CACHE[key] = _build()
    return _CACHE[key]


def kernel(x, state, node_emb, time_emb, gate_w, gate_b, gate_gamma, gate_beta,
           upd_w, upd_b, upd_gamma, upd_beta):
    global LAST_RESULT
    x = np.asarray(x, np.float32)
    state = np.asarray(state, np.float32)
    node_emb = np.asarray(node_emb, np.float32)
    time_emb = np.asarray(time_emb, np.float32)
    gate_w = np.asarray(gate_w, np.float32)
    gate_b = np.asarray(gate_b, np.float32)
    upd_w = np.asarray(upd_w, np.float32)
    upd_b = np.asarray(upd_b, np.float32)

    shared = (
        np.array_equal(np.asarray(gate_gamma), np.ones(D, np.float32))
        and np.array_equal(np.asarray(upd_gamma), np.ones(D, np.float32))
        and np.array_equal(np.asarray(gate_beta), np.zeros(D, np.float32))
        and np.array_equal(np.asarray(upd_beta), np.zeros(D, np.float32))
    )
    if not shared:
        return _np_reference(x, state, node_emb, time_emb, gate_w, gate_b,
                             gate_gamma, gate_beta, upd_w, upd_b, upd_gamma,
                             upd_beta)

    if os.environ.get("BASS_TRACE"):
        _install_prof_shim()

    from concourse.bass_utils import run_bass_kernel_spmd

    nc = _get_nc()

    xs = np.concatenate([x, state], axis=-1)  # [B, N, C]
    xq16 = np.ascontiguousarray(
        xs.reshape(B, 8, 128, C).transpose(0, 2, 1, 3)
    ).astype(np.float16)
    xTn16 = np.ascontiguousarray(x.transpose(2, 1, 0)).astype(np.float16)
    stTn16 = np.ascontiguousarray(state.transpose(2, 1, 0)).astype(np.float16)
    neT16 = np.ascontiguousarray(node_emb.T).astype(np.float16)
    teT16 = np.ascontiguousarray(time_emb.T).astype(np.float16)
    pg16 = np.ascontiguousarray(gate_w.transpose(1, 0, 3, 2)).astype(np.float16)
    pu16 = np.ascontiguousarray(upd_w.transpose(1, 0, 3, 2)).astype(np.float16)

    in_maps = []
    for c in range(NCORES):
        nlo = c * NL
        in_maps.append({
            "ne_f32": np.ascontiguousarray(node_emb[nlo : nlo + NL]),
            "neT16": np.ascontiguousarray(neT16[:, nlo : nlo + NL]),
            "te_f32": time_emb,
            "te16r": time_emb.reshape(1, B * D).astype(np.float16),
            "teT16": teT16,
            "teT4_16": np.tile(teT16, (1, 4)),
            "xq16": xq16,
            "xTn16": np.ascontiguousarray(xTn16[:, nlo : nlo + NL, :]),
            "stTn16": np.ascontiguousarray(stTn16[:, nlo : nlo + NL, :]),
            "st_loc": np.ascontiguousarray(state[:, nlo : nlo + NL, :]),
            "pg16": pg16,
            "pu16": pu16,
            "gb16": gate_b.astype(np.float16),
            "ub16": upd_b.astype(np.float16),
        })

    res = run_bass_kernel_spmd(
        nc, in_maps, list(range(NCORES)),
        trace=bool(os.environ.get("BASS_TRACE")),
    )
    LAST_RESULT = res

    h = np.empty((B, N, DO), np.float32)
    for c in range(NCORES):
        ho = res.results[c]["h_out"].reshape(4, 32, NG, DO)  # [jj, b, g, o]
        h[:, c * NL : (c + 1) * NL, :] = (
            ho.transpose(1, 2, 0, 3).reshape(B, NL, DO)
        )
    return h
